# revision 6
# baseline (speedup 1.0000x reference)
"""DMSTGCN forward on 8 Trainium2 NeuronCores (Bass/Tile).

Sharding: data-parallel over batch B=16 -> 2 batches per core; parameters
replicated. The dynamic adjacency (N=1024 x 1024 per batch) is built and kept
in SBUF; all 1x1 convs run as dense (W (x) I_13) matmuls in a transposed
"[(chan,time), node]" layout, graph hops in "[node, (chan,time)]" layout,
with PE transposes converting between the two. All matmuls use float32r
(TF32-like, 1 cyc/row at N>=256).
"""
import numpy as np

import concourse.bacc as bacc
import concourse.mybir as mybir
from concourse.tile import TileContext
from concourse.bass_utils import run_bass_kernel_spmd

F32 = mybir.dt.float32
F32R = mybir.dt.float32r
AF = mybir.ActivationFunctionType
ALU = mybir.AluOpType

B, N, T, RF = 16, 1024, 12, 13
RC, SC, DIMS, L = 16, 8, 32, 8
BN_EPS = 1e-5
NCORES = 8
BPC = B // NCORES          # batches per core
HC = 8 * RF                # 104 rows = half the (chan,time) dim
CL = RC * RF               # 208
NV_COLS = 4 + L + L * 2 * 4 + 2   # vector-pack columns

_CACHED = None             # (nc, input names) built once per process


# ----------------------------------------------------------------------------
# device kernel construction
# ----------------------------------------------------------------------------

def _build_nc():
    nc = bacc.Bacc("TRN2", target_bir_lowering=False)

    d = {}
    def din(name, shape, dt=F32R):
        d[name] = nc.dram_tensor(name, list(shape), dt, kind="ExternalInput")
        return d[name]

    din("inp", (BPC, 2, RF, N))          # padded+transposed input, f32r bits
    din("adp", (BPC, DIMS, DIMS))
    din("p2T", (DIMS, N))
    din("p3sT", (DIMS, DIMS))
    din("wstart", (2, RF, HC))
    din("wstarta", (2, RF, HC))
    din("wfc1", (2, 2, HC, HC))          # [kc, mo]
    din("wfc2", (2, 2, HC, HC))
    din("wskip", (L, 2, HC, HC))         # [i, kc]
    din("wgc", (L, 6, 2, HC, HC))        # [i, kc, mo]
    din("we1", (L, HC, 64))
    din("we2", (64, 12))
    din("iden", (128, 128))
    din("vecs", (HC, NV_COLS), F32)      # per-partition bias/scale columns
    outp = nc.dram_tensor("outp", [BPC, 12, N], F32, kind="ExternalOutput")

    with TileContext(nc) as tc, \
         tc.tile_pool(name="wp", bufs=1) as wp, \
         tc.tile_pool(name="ap", bufs=1) as ap, \
         tc.tile_pool(name="pp", bufs=1, space="PSUM") as pp:

        def wtile(name, src_ap, shape, dt=F32R):
            t = wp.tile(shape, dt, tag=name)
            nc.sync.dma_start(out=t[:], in_=src_ap)
            return t

        # ---- load parameters -------------------------------------------------
        p2T = wtile("p2T", d["p2T"][:], (DIMS, N))
        p3sT = wtile("p3sT", d["p3sT"][:], (DIMS, DIMS))
        iden = wtile("iden", d["iden"][:], (128, 128))
        vecs = wtile("vecs", d["vecs"][:], (HC, NV_COLS), F32)
        wstart = [wtile(f"wstart{h}", d["wstart"][h], (RF, HC)) for h in range(2)]
        wstarta = [wtile(f"wstarta{h}", d["wstarta"][h], (RF, HC)) for h in range(2)]
        wfc1 = [[wtile(f"wfc1_{k}{m}", d["wfc1"][k, m], (HC, HC)) for m in range(2)]
                for k in range(2)]
        wfc2 = [[wtile(f"wfc2_{k}{m}", d["wfc2"][k, m], (HC, HC)) for m in range(2)]
                for k in range(2)]
        wskip = [[wtile(f"wskip{i}_{k}", d["wskip"][i, k], (HC, HC)) for k in range(2)]
                 for i in range(L)]
        we1 = [wtile(f"we1_{i}", d["we1"][i], (HC, 64)) for i in range(L)]
        we2 = wtile("we2", d["we2"][:], (64, 12))

        # vector-pack column index map (must match _prep_host)
        vc = {}
        ci = 0
        for nm in ("sb0", "sb1", "sab0", "sab1"):
            vc[nm] = ci; ci += 1
        for i in range(L):
            vc[f"skb{i}"] = ci; ci += 1
        for i in range(L):
            for h in range(2):
                for nm in ("bns", "bnb", "bnas", "bnab"):
                    vc[f"{nm}{i}_{h}"] = ci; ci += 1
        vc["e1b"] = ci; ci += 1
        vc["e2b"] = ci; ci += 1
        assert ci == NV_COLS

        def vcol(nm, rows=HC):
            return vecs[:rows, vc[nm]:vc[nm] + 1]

        NS = (slice(0, 512), slice(512, 1024))

        for b in range(BPC):
            # ================= adjacency: AT[v, w] = A[w, v] =================
            adp = ap.tile((DIMS, DIMS), F32R, tag="adp", name=f"adp{b}")
            nc.sync.dma_start(out=adp[:], in_=d["adp"][b])

            srcT_ps = pp.tile((DIMS, N), F32, tag="pwork", bufs=2)
            for ns in NS:
                nc.tensor.matmul(srcT_ps[:, ns], adp[:], p2T[:, ns],
                                 start=True, stop=True)
            srcT = ap.tile((DIMS, N), F32R, tag="srcT")
            nc.scalar.activation(srcT[:], srcT_ps[:], AF.Copy)
            srcTn = ap.tile((DIMS, N), F32R, tag="srcTn")
            nc.scalar.activation(srcTn[:], srcT_ps[:], AF.Copy, scale=-1.0)

            u_ps = pp.tile((DIMS, N), F32, tag="pwork", bufs=2)
            for ns in NS:
                nc.tensor.matmul(u_ps[:, ns], p3sT[:], srcT[:, ns],
                                 start=True, stop=True)
            u = ap.tile((DIMS, N), F32R, tag="u")
            nc.scalar.activation(u[:], u_ps[:], AF.Copy)

            AT = [ap.tile((128, N), F32R, tag=f"AT{v}", name=f"AT{v}_{b}") for v in range(8)]
            for v in range(8):
                # accumulate D = x1^T - x1 directly in PSUM (negated weights)
                dps = pp.tile((128, N), F32, tag="pwork", bufs=2)
                cs = slice(v * 128, (v + 1) * 128)
                for ns in NS:
                    nc.tensor.matmul(dps[:, ns], u[:, cs], srcT[:, ns],
                                     start=True, stop=False)
                    nc.tensor.matmul(dps[:, ns], srcTn[:, cs], u[:, ns],
                                     start=False, stop=True)
                dt_ = ap.tile((128, N), F32, tag="Dt")
                # relu(tanh(D)) == tanh(max(D, 0))
                nc.vector.tensor_scalar(dt_[:], dps[:], 0.0, None, ALU.max)
                nc.scalar.activation(AT[v][:], dt_[:], AF.Tanh)

            # ================= start convs ===================================
            in0 = ap.tile((RF, N), F32R, tag="in0", name=f"in0_{b}")
            in1 = ap.tile((RF, N), F32R, tag="in1", name=f"in1_{b}")
            nc.sync.dma_start(out=in0[:], in_=d["inp"][b, 0])
            nc.sync.dma_start(out=in1[:], in_=d["inp"][b, 1])

            xt = [None, None]   # current x, T-layout halves [104, 1024] f32r
            xa = [None, None]   # current x_a, f32
            for h in range(2):
                ps = pp.tile((HC, N), F32, tag="pwork", bufs=2)
                for ns in NS:
                    nc.tensor.matmul(ps[:, ns], wstart[h][:], in0[:, ns],
                                     start=True, stop=True)
                xt[h] = ap.tile((HC, N), F32R, tag=f"XT{h}", bufs=2, name=f"XT{h}_{b}_init")
                nc.scalar.activation(xt[h][:], ps[:], AF.Identity, bias=vcol(f"sb{h}"))
                psa = pp.tile((HC, N), F32, tag="pwork", bufs=2)
                for ns in NS:
                    nc.tensor.matmul(psa[:, ns], wstarta[h][:], in1[:, ns],
                                     start=True, stop=True)
                xa[h] = ap.tile((HC, N), F32, tag=f"XA{h}", bufs=2, name=f"XA{h}_{b}_init")
                nc.scalar.activation(xa[h][:], psa[:], AF.Identity, bias=vcol(f"sab{h}"))

            end_ps = pp.tile((64, N), F32, tag="endps")

            # ================= layers ========================================
            for i in range(L):
                # -- channel attention + sigmoid: xn = sigmoid(2*fc2(relu(fc1(x))) + x)
                r1 = [None, None]
                for mo in range(2):
                    m1 = pp.tile((HC, N), F32, tag="pwork", bufs=2)
                    for ns in NS:
                        for kc in range(2):
                            nc.tensor.matmul(m1[:, ns], wfc1[kc][mo][:],
                                             xt[kc][:, ns],
                                             start=(kc == 0), stop=(kc == 1))
                    r1[mo] = ap.tile((HC, N), F32R, tag=f"R1_{mo}", name=f"R1_{mo}_{b}_{i}")
                    nc.scalar.activation(r1[mo][:], m1[:], AF.Relu)
                xn = [None, None]
                for mo in range(2):
                    a_ps = pp.tile((HC, N), F32, tag="pwork", bufs=2)
                    for ns in NS:
                        for kc in range(2):
                            nc.tensor.matmul(a_ps[:, ns], wfc2[kc][mo][:],
                                             r1[kc][:, ns],
                                             start=(kc == 0), stop=(kc == 1))
                    tmp = ap.tile((HC, N), F32, tag=f"tmp{mo}")
                    nc.vector.scalar_tensor_tensor(
                        tmp[:], a_ps[:], 2.0, xt[mo][:].bitcast(F32),
                        ALU.mult, ALU.add)
                    xn[mo] = ap.tile((HC, N), F32R, tag=f"XN{mo}", name=f"XN{mo}_{b}_{i}")
                    nc.scalar.activation(xn[mo][:], tmp[:], AF.Sigmoid)

                # -- skip conv -> relu -> accumulate into end1 psum
                sk_ps = pp.tile((HC, N), F32, tag="pwork", bufs=2)
                for ns in NS:
                    for kc in range(2):
                        nc.tensor.matmul(sk_ps[:, ns], wskip[i][kc][:],
                                         xn[kc][:, ns],
                                         start=(kc == 0), stop=(kc == 1))
                rsk = ap.tile((HC, N), F32R, tag="rsk")
                nc.scalar.activation(rsk[:], sk_ps[:], AF.Relu, bias=vcol(f"skb{i}"))
                for ns in NS:
                    nc.tensor.matmul(end_ps[:, ns], we1[i][:], rsk[:, ns],
                                     start=(i == 0), stop=(i == L - 1))

                # -- V-layout of xn via PE transposes
                xv = [None] * 8
                for v in range(8):
                    tp = pp.tile((128, CL), F32R, tag="ptr", bufs=2)
                    cs = slice(v * 128, (v + 1) * 128)
                    for h in range(2):
                        nc.tensor.transpose(tp[:, h * HC:(h + 1) * HC],
                                            xn[h][:, cs], iden[:HC, :HC])
                    xv[v] = ap.tile((128, CL), F32R, tag=f"XV{v}", name=f"XV{v}_{b}_{i}")
                    nc.vector.tensor_copy(xv[v][:], tp[:])

                # -- h1T = (A @ x)^T : [(c,l), w]
                h1t = [None, None]
                for mo in range(2):
                    h_ps = pp.tile((HC, N), F32, tag="pwork", bufs=2)
                    ms = slice(mo * HC, (mo + 1) * HC)
                    for ns in NS:
                        for k in range(8):
                            nc.tensor.matmul(h_ps[:, ns], xv[k][:, ms],
                                             AT[k][:, ns],
                                             start=(k == 0), stop=(k == 7))
                    h1t[mo] = ap.tile((HC, N), F32R, tag=f"H1T{mo}", name=f"H1T{mo}_{b}_{i}")
                    nc.scalar.activation(h1t[mo][:], h_ps[:], AF.Copy)

                # -- h1 V-layout
                h1v = [None] * 8
                for v in range(8):
                    tp = pp.tile((128, CL), F32R, tag="ptr", bufs=2)
                    cs = slice(v * 128, (v + 1) * 128)
                    for h in range(2):
                        nc.tensor.transpose(tp[:, h * HC:(h + 1) * HC],
                                            h1t[h][:, cs], iden[:HC, :HC])
                    h1v[v] = ap.tile((128, CL), F32R, tag=f"H1V{v}", name=f"H1V{v}_{b}_{i}")
                    nc.vector.tensor_copy(h1v[v][:], tp[:])

                # -- h2T = (A @ h1)^T
                h2t = [None, None]
                for mo in range(2):
                    h_ps = pp.tile((HC, N), F32, tag="pwork", bufs=2)
                    ms = slice(mo * HC, (mo + 1) * HC)
                    for ns in NS:
                        for k in range(8):
                            nc.tensor.matmul(h_ps[:, ns], h1v[k][:, ms],
                                             AT[k][:, ns],
                                             start=(k == 0), stop=(k == 7))
                    h2t[mo] = ap.tile((HC, N), F32R, tag=f"H2T{mo}", name=f"H2T{mo}_{b}_{i}")
                    nc.scalar.activation(h2t[mo][:], h_ps[:], AF.Copy)

                # -- gconv over h = [x, h1, h2] + residuals + batchnorm
                hcat = [xn[0], xn[1], h1t[0], h1t[1], h2t[0], h2t[1]]
                wgci = [[ap.tile((HC, HC), F32R, tag=f"wgck{k}{m}", bufs=2,
                                 name=f"wgc{b}_{i}_{k}{m}") for m in range(2)]
                        for k in range(6)]
                for k in range(6):
                    for m in range(2):
                        nc.sync.dma_start(out=wgci[k][m][:], in_=d["wgc"][i, k, m])
                for mo in range(2):
                    g_ps = pp.tile((HC, N), F32, tag="pwork", bufs=2)
                    for ns in NS:
                        for kc in range(6):
                            nc.tensor.matmul(g_ps[:, ns], wgci[kc][mo][:],
                                             hcat[kc][:, ns],
                                             start=(kc == 0), stop=(kc == 5))
                    tmp = ap.tile((HC, N), F32, tag=f"tmp{mo}")
                    nc.vector.scalar_tensor_tensor(
                        tmp[:], g_ps[:], 0.0, xa[mo][:], ALU.bypass, ALU.add)
                    nc.vector.scalar_tensor_tensor(
                        tmp[:], tmp[:], 0.0, xt[mo][:].bitcast(F32),
                        ALU.bypass, ALU.add)
                    nxt = ap.tile((HC, N), F32R, tag=f"XT{mo}", bufs=2)
                    nc.scalar.activation(nxt[:], tmp[:], AF.Identity,
                                         bias=vcol(f"bnb{i}_{mo}"),
                                         scale=vcol(f"bns{i}_{mo}"))
                    nxa = ap.tile((HC, N), F32, tag=f"XA{mo}", bufs=2)
                    nc.scalar.activation(nxa[:], xa[mo][:], AF.Identity,
                                         bias=vcol(f"bnab{i}_{mo}"),
                                         scale=vcol(f"bnas{i}_{mo}"))
                    xt[mo], xa[mo] = nxt, nxa

            # ================= end convs =====================================
            o1 = ap.tile((64, N), F32R, tag="o1")
            nc.scalar.activation(o1[:], end_ps[:], AF.Relu, bias=vcol("e1b", 64))
            o2_ps = pp.tile((12, N), F32, tag="pwork", bufs=2)
            for ns in NS:
                nc.tensor.matmul(o2_ps[:, ns], we2[:], o1[:, ns],
                                 start=True, stop=True)
            ob = ap.tile((12, N), F32, tag="ob")
            nc.scalar.activation(ob[:], o2_ps[:], AF.Identity, bias=vcol("e2b", 12))
            nc.sync.dma_start(out=outp[b], in_=ob[:])

    nc.finalize()
    return nc


# ----------------------------------------------------------------------------
# host-side preprocessing
# ----------------------------------------------------------------------------

def _kron13(w):
    """w: (o, c) block -> lhsT (c*13, o*13) = kron(w.T, I13)."""
    return np.kron(np.ascontiguousarray(w.T), np.eye(RF, dtype=np.float32)
                   ).astype(np.float32)


def _prep_host(inputs):
    f = lambda x: np.asarray(x, dtype=np.float32)
    x_in = f(inputs["inputs"])                    # (B,2,N,T)
    ind = np.asarray(inputs["ind"]).astype(np.int64)
    p1, p2, p3, pk = f(inputs["p1"]), f(inputs["p2"]), f(inputs["p3"]), f(inputs["pk"])

    xo = np.pad(x_in, ((0, 0), (0, 0), (0, 0), (RF - T, 0)))
    inp_t = np.ascontiguousarray(xo.transpose(0, 1, 3, 2))        # (B,2,RF,N)

    te = p1[ind]                                                   # (B,DIMS)
    adp = np.einsum("bi,ijk->bjk", te, pk).astype(np.float32)      # (B,j,k)

    start_w, start_b = f(inputs["start_w"]), f(inputs["start_b"])
    starta_w, starta_b = f(inputs["starta_w"]), f(inputs["starta_b"])
    fc1_w, fc2_w = f(inputs["fc1_w"]), f(inputs["fc2_w"])
    skip_w, skip_b = f(inputs["skip_w"]), f(inputs["skip_b"])
    gconv_w, gconv_b = f(inputs["gconv_w"]), f(inputs["gconv_b"])
    bn_g, bn_b = f(inputs["bn_g"]), f(inputs["bn_b"])
    bna_g, bna_b = f(inputs["bna_g"]), f(inputs["bna_b"])
    end1_w, end1_b = f(inputs["end1_w"]), f(inputs["end1_b"])
    end2_w, end2_b = f(inputs["end2_w"]), f(inputs["end2_b"])

    eye13 = np.eye(RF, dtype=np.float32)
    wstart = np.stack([np.kron(start_w[h * 8:(h + 1) * 8, 0][None, :], eye13)
                       for h in range(2)])                          # (2,13,104)
    wstarta = np.stack([np.kron(starta_w[h * 8:(h + 1) * 8, 0][None, :], eye13)
                        for h in range(2)])
    wfc1 = np.stack([np.stack([_kron13(fc1_w[m * 8:(m + 1) * 8, k * 8:(k + 1) * 8])
                               for m in range(2)]) for k in range(2)])
    wfc2 = np.stack([np.stack([_kron13(fc2_w[m * 8:(m + 1) * 8, k * 8:(k + 1) * 8])
                               for m in range(2)]) for k in range(2)])
    wskip = np.stack([np.stack([
        np.pad(_kron13(skip_w[i][:, k * 8:(k + 1) * 8]), ((0, 0), (0, 0)))
        for k in range(2)]) for i in range(L)])                     # (L,2,104,104)
    wgc = np.stack([np.stack([np.stack([
        _kron13(gconv_w[i][m * 8:(m + 1) * 8, k * 8:(k + 1) * 8])
        for m in range(2)]) for k in range(6)]) for i in range(L)])
    we1 = np.stack([np.ascontiguousarray(
        end1_w[:, (L - 1 - i) * HC:(L - i) * HC].T) for i in range(L)])
    we2 = np.ascontiguousarray(end2_w.T)                            # (64,12)

    vecs = np.zeros((HC, NV_COLS), dtype=np.float32)
    ci = 0
    for h in range(2):
        vecs[:, ci] = np.repeat(start_b[h * 8:(h + 1) * 8], RF); ci += 1
    for h in range(2):
        vecs[:, ci] = np.repeat(starta_b[h * 8:(h + 1) * 8], RF); ci += 1
    for i in range(L):
        vecs[:, ci] = np.repeat(skip_b[i], RF); ci += 1
    bns = (bn_g / np.sqrt(1.0 + BN_EPS)).astype(np.float32)
    bnas = (bna_g / np.sqrt(1.0 + BN_EPS)).astype(np.float32)
    for i in range(L):
        for h in range(2):
            s = slice(h * 8, (h + 1) * 8)
            vecs[:, ci] = np.repeat(bns[i][s], RF); ci += 1
            # gconv bias folded through the bn affine
            vecs[:, ci] = np.repeat(bn_b[i][s] + bns[i][s] * gconv_b[i][s], RF); ci += 1
            vecs[:, ci] = np.repeat(2.0 * bnas[i][s], RF); ci += 1
            vecs[:, ci] = np.repeat(bna_b[i][s], RF); ci += 1
    vecs[:64, ci] = end1_b; ci += 1
    vecs[:12, ci] = end2_b; ci += 1
    assert ci == NV_COLS

    shared = {
        "p2T": np.ascontiguousarray(p2.T),
        "p3sT": np.ascontiguousarray(p3[:DIMS, :DIMS].T),
        "wstart": wstart, "wstarta": wstarta,
        "wfc1": wfc1, "wfc2": wfc2, "wskip": wskip, "wgc": wgc,
        "we1": we1, "we2": we2,
        "iden": np.eye(128, dtype=np.float32),
        "vecs": vecs,
    }
    in_maps = []
    for c in range(NCORES):
        bs = slice(c * BPC, (c + 1) * BPC)
        m = dict(shared)
        m["inp"] = np.ascontiguousarray(inp_t[bs])
        m["adp"] = np.ascontiguousarray(adp[bs])
        in_maps.append(m)
    return in_maps


def _get_nc():
    global _CACHED
    if _CACHED is None:
        _CACHED = _build_nc()
    return _CACHED


def run(inputs, trace=False):
    nc = _get_nc()
    in_maps = _prep_host(inputs)
    res = run_bass_kernel_spmd(nc, in_maps, core_ids=list(range(NCORES)),
                               trace=trace)
    out = np.stack([res.results[c]["outp"] for c in range(NCORES)])
    out = out.reshape(B, 12, N, 1).astype(np.float32)
    return out, res


def kernel(**inputs):
    out, _ = run(inputs)
    return out


# revision 10
# speedup vs baseline: 1.1448x; 1.1448x over previous
"""DMSTGCN forward on 8 Trainium2 NeuronCores (Bass/Tile).

Sharding: data-parallel over batch B=16 -> 2 batches per core; parameters
replicated. The dynamic adjacency (1024x1024 per batch) is built and kept in
SBUF (bf16); 1x1 convs run as block-diagonal (W (x) I) matmuls in an l-major
"[(time,chan), node]" layout, graph hops in "[node, (time,chan)]" layout with
PE transposes between the two. Trunk math is float32r (TF32-like), graph-hop
operands bf16. The two batches are emitted layer-interleaved so the tensor
engine can fill the other batch's dependency stalls.
"""
import numpy as np
import ml_dtypes

import concourse.bacc as bacc
import concourse.mybir as mybir
from concourse.tile import TileContext
from concourse.bass_utils import run_bass_kernel_spmd

F32 = mybir.dt.float32
F32R = mybir.dt.float32r
BF16 = mybir.dt.bfloat16
AF = mybir.ActivationFunctionType
ALU = mybir.AluOpType

B, N, T, RF = 16, 1024, 12, 13
RC, SC, DIMS, L = 16, 8, 32, 8
BN_EPS = 1e-5
NCORES = 8
BPC = B // NCORES          # batches per core
CL = RC * RF               # 208 rows in T-layout
SKR = SC * RF              # 104 skip rows
# l-major T-layout row chunks: (offset, rows)
CH = ((0, 128), (128, 80))
NV_COLS = 4 + L + L * 2 * 3 + 2

_CACHED = None


def _build_nc():
    nc = bacc.Bacc("TRN2", target_bir_lowering=False)

    d = {}
    def din(name, shape, dt=F32R):
        d[name] = nc.dram_tensor(name, list(shape), dt, kind="ExternalInput")

    din("inp", (BPC, 2, RF, N))
    din("adp", (BPC, DIMS, DIMS))
    din("p2T", (DIMS, N))
    din("p3sT", (DIMS, DIMS))
    din("wstart0", (2, RF, 128))        # [x|xa], chunk0
    din("wstart1", (2, RF, 80))
    din("wfc1_0", (128, 128)); din("wfc1_1", (80, 80))
    din("wfc2_0", (128, 128), BF16); din("wfc2_1", (80, 80), BF16)
    din("wskip0", (L, 128, 64), BF16)
    din("wskip1", (L, 80, 40), BF16)
    din("wgc0", (L, 3, 128, 128), BF16)  # [i, src, ...]
    din("wgc1", (L, 3, 80, 80), BF16)
    din("we1", (L, SKR, 64), BF16)
    din("we2", (64, 12))
    din("iden", (128, 128))
    din("idenb", (128, 128), BF16)
    din("vecs", (128, NV_COLS), F32)
    outp = nc.dram_tensor("outp", [BPC, 12, N], F32, kind="ExternalOutput")

    with TileContext(nc) as tc, \
         tc.tile_pool(name="wp", bufs=1) as wp, \
         tc.tile_pool(name="ap", bufs=1) as ap, \
         tc.tile_pool(name="pp", bufs=1, space="PSUM") as pp:

        def wtile(name, src_ap, shape, dt=F32R):
            t = wp.tile(shape, dt, tag=name, name=name)
            nc.sync.dma_start(out=t[:], in_=src_ap)
            return t

        p2T = wtile("p2T", d["p2T"][:], (DIMS, N))
        p3sT = wtile("p3sT", d["p3sT"][:], (DIMS, DIMS))
        idenb = wtile("idenb", d["idenb"][:], (128, 128), BF16)
        vecs = wtile("vecs", d["vecs"][:], (128, NV_COLS), F32)
        wstart = [[wtile(f"wst{s}_{c}", d[f"wstart{c}"][s],
                         (RF, CH[c][1])) for c in range(2)] for s in range(2)]
        wfc1 = [wtile(f"wfc1_{c}", d[f"wfc1_{c}"][:],
                      (CH[c][1], CH[c][1])) for c in range(2)]
        wfc2 = [wtile(f"wfc2_{c}", d[f"wfc2_{c}"][:],
                      (CH[c][1], CH[c][1]), BF16) for c in range(2)]
        wskip = [[wtile(f"wsk{i}_{c}", d[f"wskip{c}"][i],
                        (CH[c][1], (64, 40)[c]), BF16) for c in range(2)]
                 for i in range(L)]
        we1 = [wtile(f"we1_{i}", d["we1"][i], (SKR, 64), BF16) for i in range(L)]
        we2 = wtile("we2", d["we2"][:], (64, 12))

        vc = {}
        ci = 0
        for nm in ("sb0", "sb1", "sab0", "sab1"):
            vc[nm] = ci; ci += 1
        for i in range(L):
            vc[f"skb{i}"] = ci; ci += 1
        for i in range(L):
            for c in range(2):
                for nm in ("bns", "bnb", "av"):
                    vc[f"{nm}{i}_{c}"] = ci; ci += 1
        vc["e1b"] = ci; ci += 1
        vc["e2b"] = ci; ci += 1
        assert ci == NV_COLS

        def vcol(nm, rows=128):
            return vecs[:rows, vc[nm]:vc[nm] + 1]

        NS = (slice(0, 512), slice(512, 1024))

        st = [dict() for _ in range(BPC)]   # per-batch tile state

        # ---------------- adjacency ----------------
        def phase0(b):
            adp = ap.tile((DIMS, DIMS), F32R, tag="adp", name=f"adp{b}")
            nc.sync.dma_start(out=adp[:], in_=d["adp"][b])
            srcT_ps = pp.tile((DIMS, N), F32, tag="pwork", bufs=2,
                              name=f"srcTps{b}")
            for ns in NS:
                nc.tensor.matmul(srcT_ps[:, ns], adp[:], p2T[:, ns],
                                 start=True, stop=True)
            srcT = ap.tile((DIMS, N), F32R, tag="srcT", name=f"srcT{b}")[:]
            srcTn = ap.tile((DIMS, N), F32R, tag="srcTn", name=f"srcTn{b}")[:]
            nc.scalar.activation(srcT, srcT_ps[:], AF.Copy)
            nc.scalar.activation(srcTn, srcT_ps[:], AF.Copy, scale=-1.0)

            u = ap.tile((DIMS, N), F32R, tag="u", name=f"u{b}")[:]
            u_ps = pp.tile((DIMS, N), F32, tag="pwork", bufs=2, name=f"ups{b}")
            for ns in NS:
                nc.tensor.matmul(u_ps[:, ns], p3sT[:], srcT[:, ns],
                                 start=True, stop=True)
            nc.scalar.activation(u, u_ps[:], AF.Copy)

            AT = [ap.tile((128, N), BF16, tag=f"AT{b}_{v}", name=f"AT{b}_{v}")
                  for v in range(8)]
            for v in range(8):
                dps = pp.tile((128, N), F32, tag="pwork", bufs=2,
                              name=f"dps{b}_{v}")
                cs = slice(v * 128, (v + 1) * 128)
                for ns in NS:
                    nc.tensor.matmul(dps[:, ns], u[:, cs], srcT[:, ns],
                                     start=True, stop=False)
                    nc.tensor.matmul(dps[:, ns], srcTn[:, cs], u[:, ns],
                                     start=False, stop=True)
                dt_ = ap.tile((128, N), BF16, tag="Dt", name=f"Dt{b}_{v}")
                # relu(tanh(x1t - x1)) == tanh(max(x1t - x1, 0))
                nc.vector.tensor_scalar(dt_[:], dps[:], 0.0, None, ALU.max)
                nc.scalar.activation(AT[v][:], dt_[:], AF.Tanh)
            st[b]["AT"] = AT

        # ---------------- start convs ----------------
        def start(b):
            in0 = ap.tile((RF, N), F32R, tag="in0", name=f"in0_{b}")[:]
            in1 = ap.tile((RF, N), F32R, tag="in1", name=f"in1_{b}")[:]
            nc.sync.dma_start(out=in0, in_=d["inp"][b, 0])
            nc.sync.dma_start(out=in1, in_=d["inp"][b, 1])
            xt, xa = [None, None], [None, None]
            for c in range(2):
                rows = CH[c][1]
                ps = pp.tile((rows, N), F32, tag="pwork", bufs=2,
                             name=f"stp{b}_{c}")
                for ns in NS:
                    nc.tensor.matmul(ps[:, ns], wstart[0][c][:], in0[:, ns],
                                     start=True, stop=True)
                xt[c] = ap.tile((rows, N), F32R, tag=f"XT{b}_{c}", bufs=2,
                                name=f"XT{b}_{c}_init")
                nc.scalar.activation(xt[c][:], ps[:], AF.Identity,
                                     bias=vcol(f"sb{c}", rows))
                psa = pp.tile((rows, N), F32, tag="pwork", bufs=2,
                              name=f"stpa{b}_{c}")
                for ns in NS:
                    nc.tensor.matmul(psa[:, ns], wstart[1][c][:], in1[:, ns],
                                     start=True, stop=True)
                xa[c] = ap.tile((rows, N), BF16, tag=f"XA{b}_{c}",
                                name=f"XA{b}_{c}")
                nc.scalar.activation(xa[c][:], psa[:], AF.Identity,
                                     bias=vcol(f"sab{c}", rows))
            st[b]["xt"], st[b]["xa"] = xt, xa
            end_sb = ap.tile((64, N), F32, tag=f"END{b}", name=f"END{b}")
            st[b]["end"] = end_sb

        # ---------------- one layer ----------------
        def layer(b, i):
            xt, xa = st[b]["xt"], st[b]["xa"]
            AT = st[b]["AT"]

            # gconv weights streamed per (b, i)
            gcw = [[ap.tile((CH[c][1], CH[c][1]), BF16, tag=f"gcw{b}_{c}_{s}",
                            bufs=2, name=f"gcw{b}_{i}_{c}_{s}")
                    for c in range(2)] for s in range(3)]
            for s in range(3):
                for c in range(2):
                    nc.sync.dma_start(out=gcw[s][c][:], in_=d[f"wgc{c}"][i, s])

            # attention + sigmoid (per-chunk independent: block-diagonal)
            xn = [None, None]
            for c in range(2):
                rows = CH[c][1]
                m1 = pp.tile((rows, N), F32, tag="pwork", bufs=2,
                             name=f"m1_{b}_{i}_{c}")
                for ns in NS:
                    nc.tensor.matmul(m1[:, ns], wfc1[c][:], xt[c][:, ns],
                                     start=True, stop=True)
                r1 = ap.tile((rows, N), BF16, tag=f"R1{b}_{c}",
                             name=f"R1{b}_{i}_{c}")
                nc.scalar.activation(r1[:], m1[:], AF.Relu)
                a_ps = pp.tile((rows, N), F32, tag="pwork", bufs=2,
                               name=f"aps{b}_{i}_{c}")
                for ns in NS:
                    nc.tensor.matmul(a_ps[:, ns], wfc2[c][:], r1[:, ns],
                                     start=True, stop=True)
                tmp = ap.tile((rows, N), F32, tag=f"tmp{b}_{c}",
                              name=f"sg{b}_{i}_{c}")
                nc.vector.scalar_tensor_tensor(
                    tmp[:], a_ps[:], 2.0, xt[c][:].bitcast(F32),
                    ALU.mult, ALU.add)
                xn[c] = ap.tile((rows, N), BF16, tag=f"XN{b}_{c}",
                                name=f"XN{b}_{i}_{c}")
                nc.scalar.activation(xn[c][:], tmp[:], AF.Sigmoid)

            # skip conv -> relu -> end1 matmul -> SBUF accumulator
            sk_ps = pp.tile((SKR, N), F32, tag="pwork", bufs=2,
                            name=f"skp{b}_{i}")
            for ns in NS:
                nc.tensor.matmul(sk_ps[:64, ns], wskip[i][0][:], xn[0][:, ns],
                                 start=True, stop=True)
                nc.tensor.matmul(sk_ps[64:, ns], wskip[i][1][:], xn[1][:, ns],
                                 start=True, stop=True)
            rsk = ap.tile((SKR, N), BF16, tag=f"rsk{b}", name=f"rsk{b}_{i}")
            nc.scalar.activation(rsk[:], sk_ps[:], AF.Relu,
                                 bias=vcol(f"skb{i}", SKR))
            e_ps = pp.tile((64, N), F32, tag="pwork", bufs=2,
                           name=f"eps{b}_{i}")
            for ns in NS:
                nc.tensor.matmul(e_ps[:, ns], we1[i][:], rsk[:, ns],
                                 start=True, stop=True)
            if i == 0:
                nc.vector.tensor_copy(st[b]["end"][:], e_ps[:])
            else:
                nc.vector.scalar_tensor_tensor(
                    st[b]["end"][:], e_ps[:], 0.0, st[b]["end"][:],
                    ALU.bypass, ALU.add)

            # V-layout of xn via PE transposes
            xv = [None] * 8
            for v in range(8):
                tp = pp.tile((128, CL), BF16, tag="ptr", bufs=3,
                             name=f"tpx{b}_{i}_{v}")
                cs = slice(v * 128, (v + 1) * 128)
                for c in range(2):
                    o, rows = CH[c]
                    nc.tensor.transpose(tp[:, o:o + rows], xn[c][:, cs],
                                        idenb[:rows, :rows])
                xv[v] = ap.tile((128, CL), BF16, tag=f"XV{b}_{v}",
                                name=f"XV{b}_{i}_{v}")
                nc.vector.tensor_copy(xv[v][:], tp[:])

            def hop(rhs_v, nm):
                """A-hop in V-orientation + transpose back to T-layout."""
                hv = [None] * 8
                for w in range(8):
                    h_ps = pp.tile((128, CL), F32, tag="ptr", bufs=3,
                                   name=f"hp{nm}{b}_{i}_{w}")
                    ws = slice(w * 128, (w + 1) * 128)
                    for k in range(8):
                        nc.tensor.matmul(h_ps[:], AT[k][:, ws], rhs_v[k][:],
                                         start=(k == 0), stop=(k == 7))
                    hv[w] = ap.tile((128, CL), BF16, tag=f"{nm}V{b}_{w}",
                                    name=f"{nm}V{b}_{i}_{w}")
                    nc.vector.tensor_copy(hv[w][:], h_ps[:])
                ht = [ap.tile((CH[c][1], N), BF16, tag=f"{nm}T{b}_{c}",
                              name=f"{nm}T{b}_{i}_{c}") for c in range(2)]
                for w in range(8):
                    tp = pp.tile((128, 256), BF16, tag="ptr", bufs=3,
                                 name=f"tp{nm}{b}_{i}_{w}")
                    for c in range(2):
                        o, rows = CH[c]
                        nc.tensor.transpose(tp[:rows, c * 128:c * 128 + 128],
                                            hv[w][:, o:o + rows],
                                            idenb[:, :])
                    ws = slice(w * 128, (w + 1) * 128)
                    for c in range(2):
                        rows = CH[c][1]
                        nc.scalar.activation(ht[c][:, ws],
                                             tp[:rows, c * 128:c * 128 + 128],
                                             AF.Copy)
                return hv, ht

            h1v, h1t = hop(xv, "H1")
            _, h2t = hop(h1v, "H2")

            # gconv (block-diag over l) + residuals + batchnorm
            srcs = (xn, h1t, h2t)
            for c in range(2):
                rows = CH[c][1]
                g_ps = pp.tile((rows, N), F32, tag="pwork", bufs=2,
                               name=f"gp{b}_{i}_{c}")
                for ns in NS:
                    for s in range(3):
                        nc.tensor.matmul(g_ps[:, ns], gcw[s][c][:],
                                         srcs[s][c][:, ns],
                                         start=(s == 0), stop=(s == 2))
                tmp = ap.tile((rows, N), F32, tag=f"tmp{b}_{c}",
                              name=f"gt{b}_{i}_{c}")
                nc.vector.scalar_tensor_tensor(
                    tmp[:], xa[c][:], vcol(f"av{i}_{c}", rows), g_ps[:],
                    ALU.mult, ALU.add)
                nc.vector.scalar_tensor_tensor(
                    tmp[:], tmp[:], 0.0, xt[c][:].bitcast(F32),
                    ALU.bypass, ALU.add)
                nxt = ap.tile((rows, N), F32R, tag=f"XT{b}_{c}", bufs=2,
                              name=f"XT{b}_{i}_{c}")
                nc.scalar.activation(nxt[:], tmp[:], AF.Identity,
                                     bias=vcol(f"bnb{i}_{c}", rows),
                                     scale=vcol(f"bns{i}_{c}", rows))
                xt[c] = nxt

        # ---------------- end convs ----------------
        def tail(b):
            o1 = ap.tile((64, N), F32R, tag="o1", name=f"o1_{b}")
            nc.scalar.activation(o1[:], st[b]["end"][:], AF.Relu,
                                 bias=vcol("e1b", 64))
            o2_ps = pp.tile((12, N), F32, tag="pwork", bufs=2, name=f"o2p{b}")
            for ns in NS:
                nc.tensor.matmul(o2_ps[:, ns], we2[:], o1[:, ns],
                                 start=True, stop=True)
            ob = ap.tile((12, N), F32, tag="ob", name=f"ob{b}")
            nc.scalar.activation(ob[:], o2_ps[:], AF.Identity,
                                 bias=vcol("e2b", 12))
            nc.sync.dma_start(out=outp[b], in_=ob[:])

        for b in range(BPC):
            phase0(b)
        for b in range(BPC):
            start(b)
        for i in range(L):
            for b in range(BPC):
                layer(b, i)
        for b in range(BPC):
            tail(b)

    nc.finalize()
    return nc


# ----------------------------------------------------------------------------
# host-side preprocessing
# ----------------------------------------------------------------------------

def _prep_host(inputs):
    f = lambda x: np.asarray(x, dtype=np.float32)
    bf = lambda x: np.ascontiguousarray(x).astype(ml_dtypes.bfloat16)
    x_in = f(inputs["inputs"])
    ind = np.asarray(inputs["ind"]).astype(np.int64)
    p1, p2, p3, pk = f(inputs["p1"]), f(inputs["p2"]), f(inputs["p3"]), f(inputs["pk"])

    xo = np.pad(x_in, ((0, 0), (0, 0), (0, 0), (RF - T, 0)))
    inp_t = np.ascontiguousarray(xo.transpose(0, 1, 3, 2))
    te = p1[ind]
    adp = np.einsum("bi,ijk->bjk", te, pk).astype(np.float32)

    start_w, start_b = f(inputs["start_w"]), f(inputs["start_b"])
    starta_w, starta_b = f(inputs["starta_w"]), f(inputs["starta_b"])
    fc1_w, fc2_w = f(inputs["fc1_w"]), f(inputs["fc2_w"])
    skip_w, skip_b = f(inputs["skip_w"]), f(inputs["skip_b"])
    gconv_w, gconv_b = f(inputs["gconv_w"]), f(inputs["gconv_b"])
    bn_g, bn_b = f(inputs["bn_g"]), f(inputs["bn_b"])
    bna_g, bna_b = f(inputs["bna_g"]), f(inputs["bna_b"])
    end1_w, end1_b = f(inputs["end1_w"]), f(inputs["end1_b"])
    end2_w, end2_b = f(inputs["end2_w"]), f(inputs["end2_b"])

    e8, e5 = np.eye(8, dtype=np.float32), np.eye(5, dtype=np.float32)
    e13 = np.eye(RF, dtype=np.float32)
    kr = lambda e, w: np.kron(e, np.ascontiguousarray(w.T)).astype(np.float32)

    wstart0 = np.stack([np.kron(e13[:, :8], w[:, 0][None, :])
                        for w in (start_w, starta_w)]).astype(np.float32)
    wstart1 = np.stack([np.kron(e13[:, 8:], w[:, 0][None, :])
                        for w in (start_w, starta_w)]).astype(np.float32)
    wgc0 = np.stack([np.stack([kr(e8, gconv_w[i][:, s * 16:(s + 1) * 16])
                               for s in range(3)]) for i in range(L)])
    wgc1 = np.stack([np.stack([kr(e5, gconv_w[i][:, s * 16:(s + 1) * 16])
                               for s in range(3)]) for i in range(L)])
    wskip0 = np.stack([kr(e8, skip_w[i]) for i in range(L)])
    wskip1 = np.stack([kr(e5, skip_w[i]) for i in range(L)])

    # end1 columns: ref skip rows are o*13+l within the (L-1-i)-th block;
    # ours are l*8+o
    we1 = np.zeros((L, SKR, 64), dtype=np.float32)
    ll, oo = np.meshgrid(np.arange(RF), np.arange(SC), indexing="ij")
    src_col = oo.ravel() * RF + ll.ravel()          # for row l*8+o
    for i in range(L):
        we1[i] = end1_w[:, (L - 1 - i) * SKR + src_col].T

    t8 = lambda v: np.tile(v, 8)
    t5 = lambda v: np.pad(np.tile(v, 5), (0, 48))
    vecs = np.zeros((128, NV_COLS), dtype=np.float32)
    ci = 0
    vecs[:, ci] = t8(start_b); ci += 1
    vecs[:80, ci] = np.tile(start_b, 5); ci += 1
    vecs[:, ci] = t8(starta_b); ci += 1
    vecs[:80, ci] = np.tile(starta_b, 5); ci += 1
    for i in range(L):
        vecs[:SKR, ci] = np.tile(skip_b[i], RF); ci += 1
    bns = (bn_g / np.sqrt(1.0 + BN_EPS)).astype(np.float32)
    bnas = (bna_g / np.sqrt(1.0 + BN_EPS)).astype(np.float32)
    av = np.ones(16, dtype=np.float32)
    bv = np.zeros(16, dtype=np.float32)
    for i in range(L):
        bnb_adj = bn_b[i] + bns[i] * (gconv_b[i] + bv)
        vecs[:, ci] = t8(bns[i]); ci += 1
        vecs[:, ci] = t8(bnb_adj); ci += 1
        vecs[:, ci] = t8(av); ci += 1
        vecs[:80, ci] = np.tile(bns[i], 5); ci += 1
        vecs[:80, ci] = np.tile(bnb_adj, 5); ci += 1
        vecs[:80, ci] = np.tile(av, 5); ci += 1
        av = 2.0 * bnas[i] * av
        bv = 2.0 * bnas[i] * bv + bna_b[i]
    vecs[:64, ci] = end1_b; ci += 1
    vecs[:12, ci] = end2_b; ci += 1
    assert ci == NV_COLS

    shared = {
        "p2T": np.ascontiguousarray(p2.T),
        "p3sT": np.ascontiguousarray(p3[:DIMS, :DIMS].T),
        "wstart0": wstart0, "wstart1": wstart1,
        "wfc1_0": kr(e8, fc1_w), "wfc1_1": kr(e5, fc1_w),
        "wfc2_0": bf(kr(e8, fc2_w)), "wfc2_1": bf(kr(e5, fc2_w)),
        "wskip0": bf(wskip0), "wskip1": bf(wskip1),
        "wgc0": bf(wgc0), "wgc1": bf(wgc1),
        "we1": bf(we1), "we2": np.ascontiguousarray(end2_w.T),
        "iden": np.eye(128, dtype=np.float32),
        "idenb": np.eye(128, dtype=ml_dtypes.bfloat16),
        "vecs": vecs,
    }
    in_maps = []
    for c in range(NCORES):
        bs = slice(c * BPC, (c + 1) * BPC)
        m = dict(shared)
        m["inp"] = np.ascontiguousarray(inp_t[bs])
        m["adp"] = np.ascontiguousarray(adp[bs])
        in_maps.append(m)
    return in_maps


def _get_nc():
    global _CACHED
    if _CACHED is None:
        _CACHED = _build_nc()
    return _CACHED


def run(inputs, trace=False):
    nc = _get_nc()
    in_maps = _prep_host(inputs)
    res = run_bass_kernel_spmd(nc, in_maps, core_ids=list(range(NCORES)),
                               trace=trace)
    out = np.stack([res.results[c]["outp"] for c in range(NCORES)])
    out = out.reshape(B, 12, N, 1).astype(np.float32)
    return out, res


def kernel(**inputs):
    out, _ = run(inputs)
    return out


# revision 12
# speedup vs baseline: 1.1760x; 1.0273x over previous
"""DMSTGCN forward on 8 Trainium2 NeuronCores (Bass/Tile).

Sharding: data-parallel over batch B=16 -> 2 batches per core; parameters
replicated. The dynamic adjacency (1024x1024 per batch) is built and kept in
SBUF (bf16); 1x1 convs run as block-diagonal (W (x) I) matmuls in an l-major
"[(time,chan), node]" layout, graph hops in "[node, (time,chan)]" layout with
PE transposes between the two. Trunk math is float32r (TF32-like), graph-hop
operands bf16. The two batches are emitted layer-interleaved, all heavy ops
are sliced per 512 nodes, and PSUM tiles are single-bank so the scheduler can
overlap the two batch streams.
"""
import numpy as np
import ml_dtypes

import concourse.bacc as bacc
import concourse.mybir as mybir
from concourse.tile import TileContext
from concourse.bass_utils import run_bass_kernel_spmd

F32 = mybir.dt.float32
F32R = mybir.dt.float32r
BF16 = mybir.dt.bfloat16
AF = mybir.ActivationFunctionType
ALU = mybir.AluOpType

B, N, T, RF = 16, 2, 1024, 12  # placeholder, fixed below
B, N, T, RF = 16, 1024, 12, 13
RC, SC, DIMS, L = 16, 8, 32, 8
BN_EPS = 1e-5
NCORES = 8
BPC = B // NCORES          # batches per core
CL = RC * RF               # 208 rows in T-layout
SKR = SC * RF              # 104 skip rows
CH = ((0, 128), (128, 80))  # l-major T-layout row chunks
NV_COLS = 4 + L + L * 2 * 3 + 2

_CACHED = None


def _build_nc():
    nc = bacc.Bacc("TRN2", target_bir_lowering=False)

    d = {}
    def din(name, shape, dt=F32R):
        d[name] = nc.dram_tensor(name, list(shape), dt, kind="ExternalInput")

    din("inp", (BPC, 2, RF, N))
    din("adp", (BPC, DIMS, DIMS))
    din("p2T", (DIMS, N))
    din("p3sT", (DIMS, DIMS))
    din("wstart0", (2, RF, 128))
    din("wstart1", (2, RF, 80))
    din("wfc1_0", (128, 128)); din("wfc1_1", (80, 80))
    din("wfc2_0", (128, 128), BF16); din("wfc2_1", (80, 80), BF16)
    din("wskip0", (L, 128, 64), BF16)
    din("wskip1", (L, 80, 40), BF16)
    din("wgc0", (L, 3, 128, 128), BF16)
    din("wgc1", (L, 3, 80, 80), BF16)
    din("we1", (L, SKR, 64), BF16)
    din("we2", (64, 12))
    din("idenb", (128, 128), BF16)
    din("vecs", (128, NV_COLS), F32)
    outp = nc.dram_tensor("outp", [BPC, 12, N], F32, kind="ExternalOutput")

    with TileContext(nc) as tc, \
         tc.tile_pool(name="wp", bufs=1) as wp, \
         tc.tile_pool(name="ap", bufs=1) as ap, \
         tc.tile_pool(name="pp", bufs=1, space="PSUM") as pp:

        def wtile(name, src_ap, shape, dt=F32R, eng=None):
            t = wp.tile(shape, dt, tag=name, name=name)
            (eng or nc.gpsimd).dma_start(out=t[:], in_=src_ap)
            return t

        # phase0-critical loads go first on the SP queue; bulk weights on
        # gpsimd so PE can start within ~2us.
        p2T = wtile("p2T", d["p2T"][:], (DIMS, N), eng=nc.sync)
        p3sT = wtile("p3sT", d["p3sT"][:], (DIMS, DIMS), eng=nc.sync)
        adps = [wtile(f"adp{b}", d["adp"][b], (DIMS, DIMS), eng=nc.sync)
                for b in range(BPC)]

        idenb = wtile("idenb", d["idenb"][:], (128, 128), BF16)
        vecs = wtile("vecs", d["vecs"][:], (128, NV_COLS), F32)
        wstart = [[wtile(f"wst{s}_{c}", d[f"wstart{c}"][s],
                         (RF, CH[c][1])) for c in range(2)] for s in range(2)]
        wfc1 = [wtile(f"wfc1_{c}", d[f"wfc1_{c}"][:],
                      (CH[c][1], CH[c][1])) for c in range(2)]
        wfc2 = [wtile(f"wfc2_{c}", d[f"wfc2_{c}"][:],
                      (CH[c][1], CH[c][1]), BF16) for c in range(2)]
        wskip = [[wtile(f"wsk{i}_{c}", d[f"wskip{c}"][i],
                        (CH[c][1], (64, 40)[c]), BF16) for c in range(2)]
                 for i in range(L)]
        we1 = [wtile(f"we1_{i}", d["we1"][i], (SKR, 64), BF16) for i in range(L)]
        we2 = wtile("we2", d["we2"][:], (64, 12))

        vc = {}
        ci = 0
        for nm in ("sb0", "sb1", "sab0", "sab1"):
            vc[nm] = ci; ci += 1
        for i in range(L):
            vc[f"skb{i}"] = ci; ci += 1
        for i in range(L):
            for c in range(2):
                for nm in ("bns", "bnb", "av"):
                    vc[f"{nm}{i}_{c}"] = ci; ci += 1
        vc["e1b"] = ci; ci += 1
        vc["e2b"] = ci; ci += 1
        assert ci == NV_COLS

        def vcol(nm, rows=128):
            return vecs[:rows, vc[nm]:vc[nm] + 1]

        NS = (slice(0, 512), slice(512, 1024))

        st = [dict() for _ in range(BPC)]

        # ---------------- adjacency ----------------
        def phase0(b):
            adp = adps[b]
            srcT = ap.tile((DIMS, N), F32R, tag="srcT", name=f"srcT{b}")[:]
            srcTn = ap.tile((DIMS, N), F32R, tag="srcTn", name=f"srcTn{b}")[:]
            u = ap.tile((DIMS, N), F32R, tag="u", name=f"u{b}")[:]
            for nsi, ns in enumerate(NS):
                srcT_ps = pp.tile((DIMS, 512), F32, tag="pwork", bufs=3,
                                  name=f"srcTps{b}_{nsi}")
                nc.tensor.matmul(srcT_ps[:], adp[:], p2T[:, ns],
                                 start=True, stop=True)
                nc.scalar.activation(srcT[:, ns], srcT_ps[:], AF.Copy)
                nc.scalar.activation(srcTn[:, ns], srcT_ps[:], AF.Copy,
                                     scale=-1.0)
            for nsi, ns in enumerate(NS):
                u_ps = pp.tile((DIMS, 512), F32, tag="pwork", bufs=3,
                               name=f"ups{b}_{nsi}")
                nc.tensor.matmul(u_ps[:], p3sT[:], srcT[:, ns],
                                 start=True, stop=True)
                nc.scalar.activation(u[:, ns], u_ps[:], AF.Copy)

            AT = [ap.tile((128, N), BF16, tag=f"AT{b}_{v}", name=f"AT{b}_{v}")
                  for v in range(8)]
            for v in range(8):
                cs = slice(v * 128, (v + 1) * 128)
                dt_ = ap.tile((128, N), BF16, tag="Dt", name=f"Dt{b}_{v}")
                for nsi, ns in enumerate(NS):
                    dps = pp.tile((128, 512), F32, tag="pwork", bufs=3,
                                  name=f"dps{b}_{v}_{nsi}")
                    nc.tensor.matmul(dps[:], u[:, cs], srcT[:, ns],
                                     start=True, stop=False)
                    nc.tensor.matmul(dps[:], srcTn[:, cs], u[:, ns],
                                     start=False, stop=True)
                    # relu(tanh(x1t - x1)) == tanh(max(x1t - x1, 0))
                    nc.vector.tensor_scalar(dt_[:, ns], dps[:], 0.0, None,
                                            ALU.max)
                    nc.scalar.activation(AT[v][:, ns], dt_[:, ns], AF.Tanh)
            st[b]["AT"] = AT

        # ---------------- start convs ----------------
        def start(b):
            in0 = ap.tile((RF, N), F32R, tag="in0", name=f"in0_{b}")[:]
            in1 = ap.tile((RF, N), F32R, tag="in1", name=f"in1_{b}")[:]
            nc.sync.dma_start(out=in0, in_=d["inp"][b, 0])
            nc.sync.dma_start(out=in1, in_=d["inp"][b, 1])
            xt, xa = [None, None], [None, None]
            for c in range(2):
                rows = CH[c][1]
                xt[c] = ap.tile((rows, N), F32R, tag=f"XT{b}_{c}", bufs=2,
                                name=f"XT{b}_{c}_init")
                xa[c] = ap.tile((rows, N), BF16, tag=f"XA{b}_{c}",
                                name=f"XA{b}_{c}")
                for nsi, ns in enumerate(NS):
                    ps = pp.tile((rows, 512), F32, tag="pwork", bufs=3,
                                 name=f"stp{b}_{c}_{nsi}")
                    nc.tensor.matmul(ps[:], wstart[0][c][:], in0[:, ns],
                                     start=True, stop=True)
                    nc.scalar.activation(xt[c][:, ns], ps[:], AF.Identity,
                                         bias=vcol(f"sb{c}", rows))
                    psa = pp.tile((rows, 512), F32, tag="pwork", bufs=3,
                                  name=f"stpa{b}_{c}_{nsi}")
                    nc.tensor.matmul(psa[:], wstart[1][c][:], in1[:, ns],
                                     start=True, stop=True)
                    nc.scalar.activation(xa[c][:, ns], psa[:], AF.Identity,
                                         bias=vcol(f"sab{c}", rows))
            st[b]["xt"], st[b]["xa"] = xt, xa
            st[b]["end"] = ap.tile((64, N), F32, tag=f"END{b}", name=f"END{b}")

        # ---------------- one layer ----------------
        def layer(b, i):
            xt, xa = st[b]["xt"], st[b]["xa"]
            AT = st[b]["AT"]

            gcw = [[ap.tile((CH[c][1], CH[c][1]), BF16, tag=f"gcw{b}_{c}_{s}",
                            bufs=2, name=f"gcw{b}_{i}_{c}_{s}")
                    for c in range(2)] for s in range(3)]
            for s in range(3):
                for c in range(2):
                    nc.gpsimd.dma_start(out=gcw[s][c][:], in_=d[f"wgc{c}"][i, s])

            # attention + sigmoid, per chunk x per 512-slice
            xn = [None, None]
            for c in range(2):
                rows = CH[c][1]
                r1 = ap.tile((rows, N), BF16, tag=f"R1{b}_{c}",
                             name=f"R1{b}_{i}_{c}")
                sg = ap.tile((rows, N), F32, tag=f"tmp{b}_{c}",
                             name=f"sg{b}_{i}_{c}")
                xn[c] = ap.tile((rows, N), BF16, tag=f"XN{b}_{c}",
                                name=f"XN{b}_{i}_{c}")
                for nsi, ns in enumerate(NS):
                    m1 = pp.tile((rows, 512), F32, tag="pwork", bufs=3,
                                 name=f"m1_{b}_{i}_{c}_{nsi}")
                    nc.tensor.matmul(m1[:], wfc1[c][:], xt[c][:, ns],
                                     start=True, stop=True)
                    nc.scalar.activation(r1[:, ns], m1[:], AF.Relu)
                    a_ps = pp.tile((rows, 512), F32, tag="pwork", bufs=3,
                                   name=f"aps{b}_{i}_{c}_{nsi}")
                    nc.tensor.matmul(a_ps[:], wfc2[c][:], r1[:, ns],
                                     start=True, stop=True)
                    nc.vector.scalar_tensor_tensor(
                        sg[:, ns], a_ps[:], 2.0, xt[c][:, ns].bitcast(F32),
                        ALU.mult, ALU.add)
                    nc.scalar.activation(xn[c][:, ns], sg[:, ns], AF.Sigmoid)

            # skip conv -> relu -> end1 matmul -> SBUF accumulator
            rsk = ap.tile((SKR, N), BF16, tag=f"rsk{b}", name=f"rsk{b}_{i}")
            for nsi, ns in enumerate(NS):
                sk_ps = pp.tile((SKR, 512), F32, tag="pwork", bufs=3,
                                name=f"skp{b}_{i}_{nsi}")
                nc.tensor.matmul(sk_ps[:64], wskip[i][0][:], xn[0][:, ns],
                                 start=True, stop=True)
                nc.tensor.matmul(sk_ps[64:], wskip[i][1][:], xn[1][:, ns],
                                 start=True, stop=True)
                nc.scalar.activation(rsk[:, ns], sk_ps[:], AF.Relu,
                                     bias=vcol(f"skb{i}", SKR))
                e_ps = pp.tile((64, 512), F32, tag="pwork", bufs=3,
                               name=f"eps{b}_{i}_{nsi}")
                nc.tensor.matmul(e_ps[:], we1[i][:], rsk[:, ns],
                                 start=True, stop=True)
                if i == 0:
                    nc.vector.tensor_copy(st[b]["end"][:, ns], e_ps[:])
                else:
                    nc.vector.scalar_tensor_tensor(
                        st[b]["end"][:, ns], e_ps[:], 0.0,
                        st[b]["end"][:, ns], ALU.bypass, ALU.add)

            # V-layout of xn via PE transposes
            xv = [None] * 8
            for v in range(8):
                tp = pp.tile((128, CL), BF16, tag="ptr", bufs=3,
                             name=f"tpx{b}_{i}_{v}")
                cs = slice(v * 128, (v + 1) * 128)
                for c in range(2):
                    o, rows = CH[c]
                    nc.tensor.transpose(tp[:, o:o + rows], xn[c][:, cs],
                                        idenb[:rows, :rows])
                xv[v] = ap.tile((128, CL), BF16, tag=f"XV{b}_{v}",
                                name=f"XV{b}_{i}_{v}")
                nc.vector.tensor_copy(xv[v][:], tp[:])

            def hop(rv, nm):
                """A-hop in V-orientation (w-pairs) + transpose to T-layout."""
                hvp = [None] * 4
                for p in range(4):
                    h_ps = pp.tile((128, 2 * CL), F32, tag="ptr", bufs=3,
                                   name=f"hp{nm}{b}_{i}_{p}")
                    for half in range(2):
                        w = 2 * p + half
                        ws = slice(w * 128, (w + 1) * 128)
                        dst = h_ps[:, half * CL:(half + 1) * CL]
                        for k in range(8):
                            nc.tensor.matmul(dst, AT[k][:, ws], rv(k),
                                             start=(k == 0), stop=(k == 7))
                    hvp[p] = ap.tile((128, 2 * CL), BF16, tag=f"{nm}V{b}_{p}",
                                     name=f"{nm}V{b}_{i}_{p}")
                    nc.vector.tensor_copy(hvp[p][:], h_ps[:])

                ht = [ap.tile((CH[c][1], N), BF16, tag=f"{nm}T{b}_{c}",
                              name=f"{nm}T{b}_{i}_{c}") for c in range(2)]
                tpb = [pp.tile((CH[c][1], N), BF16, tag=f"ptb{c}", bufs=1,
                               name=f"tpb{nm}{b}_{i}_{c}") for c in range(2)]
                for w in range(8):
                    src = hvp[w // 2][:, (w % 2) * CL:(w % 2) * CL + CL]
                    for c in range(2):
                        o, rows = CH[c]
                        nc.tensor.transpose(tpb[c][:, w * 128:(w + 1) * 128],
                                            src[:, o:o + rows], idenb[:, :])
                    if w % 4 == 3:   # evict per 512-col half
                        half = slice((w - 3) * 128, (w + 1) * 128)
                        for c in range(2):
                            nc.scalar.activation(ht[c][:, half],
                                                 tpb[c][:, half], AF.Copy)
                return hvp, ht

            h1vp, h1t = hop(lambda k: xv[k][:], "H1")
            _, h2t = hop(
                lambda k: h1vp[k // 2][:, (k % 2) * CL:(k % 2) * CL + CL],
                "H2")

            # gconv (block-diag over l) + residuals + batchnorm
            srcs = (xn, h1t, h2t)
            for c in range(2):
                rows = CH[c][1]
                gt = ap.tile((rows, N), F32, tag=f"tmp{b}_{c}",
                             name=f"gt{b}_{i}_{c}")
                nxt = ap.tile((rows, N), F32R, tag=f"XT{b}_{c}", bufs=2,
                              name=f"XT{b}_{i}_{c}")
                for nsi, ns in enumerate(NS):
                    g_ps = pp.tile((rows, 512), F32, tag="pwork", bufs=3,
                                   name=f"gp{b}_{i}_{c}_{nsi}")
                    for s in range(3):
                        nc.tensor.matmul(g_ps[:], gcw[s][c][:],
                                         srcs[s][c][:, ns],
                                         start=(s == 0), stop=(s == 2))
                    nc.vector.scalar_tensor_tensor(
                        gt[:, ns], xa[c][:, ns], vcol(f"av{i}_{c}", rows),
                        g_ps[:], ALU.mult, ALU.add)
                    nc.gpsimd.tensor_tensor(
                        gt[:, ns], gt[:, ns], xt[c][:, ns].bitcast(F32),
                        ALU.add)
                    nc.scalar.activation(nxt[:, ns], gt[:, ns], AF.Identity,
                                         bias=vcol(f"bnb{i}_{c}", rows),
                                         scale=vcol(f"bns{i}_{c}", rows))
                xt[c] = nxt

        # ---------------- end convs ----------------
        def tail(b):
            o1 = ap.tile((64, N), F32R, tag="o1", name=f"o1_{b}")
            ob = ap.tile((12, N), F32, tag="ob", name=f"ob{b}")
            for nsi, ns in enumerate(NS):
                nc.scalar.activation(o1[:, ns], st[b]["end"][:, ns], AF.Relu,
                                     bias=vcol("e1b", 64))
                o2_ps = pp.tile((12, 512), F32, tag="pwork", bufs=3,
                                name=f"o2p{b}_{nsi}")
                nc.tensor.matmul(o2_ps[:], we2[:], o1[:, ns],
                                 start=True, stop=True)
                nc.scalar.activation(ob[:, ns], o2_ps[:], AF.Identity,
                                     bias=vcol("e2b", 12))
            nc.sync.dma_start(out=outp[b], in_=ob[:])

        for b in range(BPC):
            phase0(b)
        for b in range(BPC):
            start(b)
        for i in range(L):
            for b in range(BPC):
                layer(b, i)
        for b in range(BPC):
            tail(b)

    nc.finalize()
    return nc


# ----------------------------------------------------------------------------
# host-side preprocessing
# ----------------------------------------------------------------------------

def _prep_host(inputs):
    f = lambda x: np.asarray(x, dtype=np.float32)
    bf = lambda x: np.ascontiguousarray(x).astype(ml_dtypes.bfloat16)
    x_in = f(inputs["inputs"])
    ind = np.asarray(inputs["ind"]).astype(np.int64)
    p1, p2, p3, pk = f(inputs["p1"]), f(inputs["p2"]), f(inputs["p3"]), f(inputs["pk"])

    xo = np.pad(x_in, ((0, 0), (0, 0), (0, 0), (RF - T, 0)))
    inp_t = np.ascontiguousarray(xo.transpose(0, 1, 3, 2))
    te = p1[ind]
    adp = np.einsum("bi,ijk->bjk", te, pk).astype(np.float32)

    start_w, start_b = f(inputs["start_w"]), f(inputs["start_b"])
    starta_w, starta_b = f(inputs["starta_w"]), f(inputs["starta_b"])
    fc1_w, fc2_w = f(inputs["fc1_w"]), f(inputs["fc2_w"])
    skip_w, skip_b = f(inputs["skip_w"]), f(inputs["skip_b"])
    gconv_w, gconv_b = f(inputs["gconv_w"]), f(inputs["gconv_b"])
    bn_g, bn_b = f(inputs["bn_g"]), f(inputs["bn_b"])
    bna_g, bna_b = f(inputs["bna_g"]), f(inputs["bna_b"])
    end1_w, end1_b = f(inputs["end1_w"]), f(inputs["end1_b"])
    end2_w, end2_b = f(inputs["end2_w"]), f(inputs["end2_b"])

    e8, e5 = np.eye(8, dtype=np.float32), np.eye(5, dtype=np.float32)
    e13 = np.eye(RF, dtype=np.float32)
    kr = lambda e, w: np.kron(e, np.ascontiguousarray(w.T)).astype(np.float32)

    wstart0 = np.stack([np.kron(e13[:, :8], w[:, 0][None, :])
                        for w in (start_w, starta_w)]).astype(np.float32)
    wstart1 = np.stack([np.kron(e13[:, 8:], w[:, 0][None, :])
                        for w in (start_w, starta_w)]).astype(np.float32)
    wgc0 = np.stack([np.stack([kr(e8, gconv_w[i][:, s * 16:(s + 1) * 16])
                               for s in range(3)]) for i in range(L)])
    wgc1 = np.stack([np.stack([kr(e5, gconv_w[i][:, s * 16:(s + 1) * 16])
                               for s in range(3)]) for i in range(L)])
    wskip0 = np.stack([kr(e8, skip_w[i]) for i in range(L)])
    wskip1 = np.stack([kr(e5, skip_w[i]) for i in range(L)])

    # end1 columns: ref skip rows are o*13+l within the (L-1-i)-th block;
    # ours are l*8+o
    we1 = np.zeros((L, SKR, 64), dtype=np.float32)
    ll, oo = np.meshgrid(np.arange(RF), np.arange(SC), indexing="ij")
    src_col = oo.ravel() * RF + ll.ravel()
    for i in range(L):
        we1[i] = end1_w[:, (L - 1 - i) * SKR + src_col].T

    t8 = lambda v: np.tile(v, 8)
    vecs = np.zeros((128, NV_COLS), dtype=np.float32)
    ci = 0
    vecs[:, ci] = t8(start_b); ci += 1
    vecs[:80, ci] = np.tile(start_b, 5); ci += 1
    vecs[:, ci] = t8(starta_b); ci += 1
    vecs[:80, ci] = np.tile(starta_b, 5); ci += 1
    for i in range(L):
        vecs[:SKR, ci] = np.tile(skip_b[i], RF); ci += 1
    bns = (bn_g / np.sqrt(1.0 + BN_EPS)).astype(np.float32)
    bnas = (bna_g / np.sqrt(1.0 + BN_EPS)).astype(np.float32)
    av = np.ones(16, dtype=np.float32)
    bv = np.zeros(16, dtype=np.float32)
    for i in range(L):
        bnb_adj = bn_b[i] + bns[i] * (gconv_b[i] + bv)
        vecs[:, ci] = t8(bns[i]); ci += 1
        vecs[:, ci] = t8(bnb_adj); ci += 1
        vecs[:, ci] = t8(av); ci += 1
        vecs[:80, ci] = np.tile(bns[i], 5); ci += 1
        vecs[:80, ci] = np.tile(bnb_adj, 5); ci += 1
        vecs[:80, ci] = np.tile(av, 5); ci += 1
        av = 2.0 * bnas[i] * av
        bv = 2.0 * bnas[i] * bv + bna_b[i]
    vecs[:64, ci] = end1_b; ci += 1
    vecs[:12, ci] = end2_b; ci += 1
    assert ci == NV_COLS

    shared = {
        "p2T": np.ascontiguousarray(p2.T),
        "p3sT": np.ascontiguousarray(p3[:DIMS, :DIMS].T),
        "wstart0": wstart0, "wstart1": wstart1,
        "wfc1_0": kr(e8, fc1_w), "wfc1_1": kr(e5, fc1_w),
        "wfc2_0": bf(kr(e8, fc2_w)), "wfc2_1": bf(kr(e5, fc2_w)),
        "wskip0": bf(wskip0), "wskip1": bf(wskip1),
        "wgc0": bf(wgc0), "wgc1": bf(wgc1),
        "we1": bf(we1), "we2": np.ascontiguousarray(end2_w.T),
        "idenb": np.eye(128, dtype=ml_dtypes.bfloat16),
        "vecs": vecs,
    }
    in_maps = []
    for c in range(NCORES):
        bs = slice(c * BPC, (c + 1) * BPC)
        m = dict(shared)
        m["inp"] = np.ascontiguousarray(inp_t[bs])
        m["adp"] = np.ascontiguousarray(adp[bs])
        in_maps.append(m)
    return in_maps


def _get_nc():
    global _CACHED
    if _CACHED is None:
        _CACHED = _build_nc()
    return _CACHED


def run(inputs, trace=False):
    nc = _get_nc()
    in_maps = _prep_host(inputs)
    res = run_bass_kernel_spmd(nc, in_maps, core_ids=list(range(NCORES)),
                               trace=trace)
    out = np.stack([res.results[c]["outp"] for c in range(NCORES)])
    out = out.reshape(B, 12, N, 1).astype(np.float32)
    return out, res


def kernel(**inputs):
    out, _ = run(inputs)
    return out


# revision 16
# speedup vs baseline: 1.2950x; 1.1012x over previous
"""DMSTGCN forward on 8 Trainium2 NeuronCores (Bass/Tile).

Sharding: data-parallel over batch B=16 -> 2 batches per core; parameters
replicated. The dynamic adjacency (1024x1024 per batch) is built and kept in
SBUF (bf16); 1x1 convs run as block-diagonal (W (x) I) matmuls in an l-major
"[(time,chan), node]" layout, graph hops in "[node, (time,chan)]" layout with
PE transposes between the two. Trunk math is float32r (TF32-like), graph-hop
operands bf16. The two batches are emitted layer-interleaved, all heavy ops
are sliced per 512 nodes, and PSUM tiles are single-bank so the scheduler can
overlap the two batch streams.
"""
import numpy as np
import ml_dtypes

import concourse.bacc as bacc
import concourse.mybir as mybir
from concourse.tile import TileContext
from concourse.bass_utils import run_bass_kernel_spmd

F32 = mybir.dt.float32
F32R = mybir.dt.float32r
BF16 = mybir.dt.bfloat16
AF = mybir.ActivationFunctionType
ALU = mybir.AluOpType

B, N, T, RF = 16, 2, 1024, 12  # placeholder, fixed below
B, N, T, RF = 16, 1024, 12, 13
RC, SC, DIMS, L = 16, 8, 32, 8
BN_EPS = 1e-5
NCORES = 8
BPC = B // NCORES          # batches per core
CL = RC * RF               # 208 rows in T-layout
SKR = SC * RF              # 104 skip rows
CH = ((0, 128), (128, 80))  # l-major T-layout row chunks
NV_COLS = 4 + L + L * 2 * 3 + 2

_CACHED = None


def _build_nc():
    nc = bacc.Bacc("TRN2", target_bir_lowering=False)

    d = {}
    def din(name, shape, dt=F32R):
        d[name] = nc.dram_tensor(name, list(shape), dt, kind="ExternalInput")

    din("inp", (BPC, 2, RF, N))
    din("adp", (BPC, DIMS, DIMS))
    din("p2T", (DIMS, N))
    din("p3sT", (DIMS, DIMS))
    din("wstart0", (2, RF, 128))
    din("wstart1", (2, RF, 80))
    din("wfc1_0", (128, 128)); din("wfc1_1", (80, 80))
    din("wfc2_0", (128, 128), BF16); din("wfc2_1", (80, 80), BF16)
    din("wskip0", (L, 128, 64), BF16)
    din("wskip1", (L, 80, 40), BF16)
    din("wgc0", (L, 3, 128, 128), BF16)
    din("wgc1", (L, 3, 80, 80), BF16)
    din("we1", (L, SKR, 64), BF16)
    din("we2", (64, 12))
    din("idenb", (128, 128), BF16)
    din("vecs", (128, NV_COLS), F32)
    outp = nc.dram_tensor("outp", [BPC, 12, N], F32, kind="ExternalOutput")

    with TileContext(nc) as tc, \
         tc.tile_pool(name="wp", bufs=1) as wp, \
         tc.tile_pool(name="ap", bufs=1) as ap, \
         tc.tile_pool(name="pp", bufs=1, space="PSUM") as pp:

        def wtile(name, src_ap, shape, dt=F32R, eng=None):
            t = wp.tile(shape, dt, tag=name, name=name)
            (eng or nc.sync).dma_start(out=t[:], in_=src_ap)
            return t

        # phase0-critical loads go first on the SP queue; bulk weights on
        # gpsimd so PE can start within ~2us.
        p2T = wtile("p2T", d["p2T"][:], (DIMS, N), eng=nc.sync)
        p3sT = wtile("p3sT", d["p3sT"][:], (DIMS, DIMS), eng=nc.sync)
        adps = [wtile(f"adp{b}", d["adp"][b], (DIMS, DIMS), eng=nc.sync)
                for b in range(BPC)]

        idenb = wtile("idenb", d["idenb"][:], (128, 128), BF16)
        vecs = wtile("vecs", d["vecs"][:], (128, NV_COLS), F32)
        wstart = [[wtile(f"wst{s}_{c}", d[f"wstart{c}"][s],
                         (RF, CH[c][1])) for c in range(2)] for s in range(2)]
        wfc1 = [wtile(f"wfc1_{c}", d[f"wfc1_{c}"][:],
                      (CH[c][1], CH[c][1])) for c in range(2)]
        wfc2 = [wtile(f"wfc2_{c}", d[f"wfc2_{c}"][:],
                      (CH[c][1], CH[c][1]), BF16) for c in range(2)]
        wskip = [[wtile(f"wsk{i}_{c}", d[f"wskip{c}"][i],
                        (CH[c][1], (64, 40)[c]), BF16) for c in range(2)]
                 for i in range(L)]
        we1 = [wtile(f"we1_{i}", d["we1"][i], (SKR, 64), BF16) for i in range(L)]
        we2 = wtile("we2", d["we2"][:], (64, 12))

        vc = {}
        ci = 0
        for nm in ("sb0", "sb1", "sab0", "sab1"):
            vc[nm] = ci; ci += 1
        for i in range(L):
            vc[f"skb{i}"] = ci; ci += 1
        for i in range(L):
            for c in range(2):
                for nm in ("bns", "bnb", "av"):
                    vc[f"{nm}{i}_{c}"] = ci; ci += 1
        vc["e1b"] = ci; ci += 1
        vc["e2b"] = ci; ci += 1
        assert ci == NV_COLS

        def vcol(nm, rows=128):
            return vecs[:rows, vc[nm]:vc[nm] + 1]

        NS = (slice(0, 512), slice(512, 1024))

        st = [dict() for _ in range(BPC)]

        # ---------------- adjacency ----------------
        def phase0(b):
            adp = adps[b]
            srcT = ap.tile((DIMS, N), F32R, tag="srcT", name=f"srcT{b}")[:]
            srcTn = ap.tile((DIMS, N), F32R, tag="srcTn", name=f"srcTn{b}")[:]
            u = ap.tile((DIMS, N), F32R, tag="u", name=f"u{b}")[:]
            for nsi, ns in enumerate(NS):
                srcT_ps = pp.tile((DIMS, 512), F32, tag="pwork", bufs=3,
                                  name=f"srcTps{b}_{nsi}")
                nc.tensor.matmul(srcT_ps[:], adp[:], p2T[:, ns],
                                 start=True, stop=True)
                nc.scalar.activation(srcT[:, ns], srcT_ps[:], AF.Copy)
                nc.scalar.activation(srcTn[:, ns], srcT_ps[:], AF.Copy,
                                     scale=-1.0)
            for nsi, ns in enumerate(NS):
                u_ps = pp.tile((DIMS, 512), F32, tag="pwork", bufs=3,
                               name=f"ups{b}_{nsi}")
                nc.tensor.matmul(u_ps[:], p3sT[:], srcT[:, ns],
                                 start=True, stop=True)
                nc.scalar.activation(u[:, ns], u_ps[:], AF.Copy)

            AT = [ap.tile((128, N), BF16, tag=f"AT{b}_{v}", name=f"AT{b}_{v}")
                  for v in range(8)]
            for v in range(8):
                cs = slice(v * 128, (v + 1) * 128)
                dt_ = ap.tile((128, N), BF16, tag="Dt", name=f"Dt{b}_{v}")
                for nsi, ns in enumerate(NS):
                    dps = pp.tile((128, 512), F32, tag="pwork", bufs=3,
                                  name=f"dps{b}_{v}_{nsi}")
                    nc.tensor.matmul(dps[:], u[:, cs], srcT[:, ns],
                                     start=True, stop=False)
                    nc.tensor.matmul(dps[:], srcTn[:, cs], u[:, ns],
                                     start=False, stop=True)
                    # relu(tanh(x1t - x1)) == tanh(max(x1t - x1, 0))
                    nc.vector.tensor_scalar(dt_[:, ns], dps[:], 0.0, None,
                                            ALU.max)
                    nc.scalar.activation(AT[v][:, ns], dt_[:, ns], AF.Tanh)
            st[b]["AT"] = AT

        # ---------------- start convs ----------------
        def start(b):
            in0 = ap.tile((RF, N), F32R, tag="in0", name=f"in0_{b}")[:]
            in1 = ap.tile((RF, N), F32R, tag="in1", name=f"in1_{b}")[:]
            nc.sync.dma_start(out=in0, in_=d["inp"][b, 0])
            nc.sync.dma_start(out=in1, in_=d["inp"][b, 1])
            xt, xa = [None, None], [None, None]
            for c in range(2):
                rows = CH[c][1]
                xt[c] = ap.tile((rows, N), F32R, tag=f"XT{b}_{c}", bufs=2,
                                name=f"XT{b}_{c}_init")
                xa[c] = ap.tile((rows, N), BF16, tag=f"XA{b}_{c}",
                                name=f"XA{b}_{c}")
                for nsi, ns in enumerate(NS):
                    ps = pp.tile((rows, 512), F32, tag="pwork", bufs=3,
                                 name=f"stp{b}_{c}_{nsi}")
                    nc.tensor.matmul(ps[:], wstart[0][c][:], in0[:, ns],
                                     start=True, stop=True)
                    nc.scalar.activation(xt[c][:, ns], ps[:], AF.Identity,
                                         bias=vcol(f"sb{c}", rows))
                    psa = pp.tile((rows, 512), F32, tag="pwork", bufs=3,
                                  name=f"stpa{b}_{c}_{nsi}")
                    nc.tensor.matmul(psa[:], wstart[1][c][:], in1[:, ns],
                                     start=True, stop=True)
                    nc.scalar.activation(xa[c][:, ns], psa[:], AF.Identity,
                                         bias=vcol(f"sab{c}", rows))
            st[b]["xt"], st[b]["xa"] = xt, xa
            st[b]["end"] = ap.tile((64, N), F32, tag=f"END{b}", name=f"END{b}")

        # ---------------- one layer, both batches stage-interleaved ----------
        def layer_pair(i):
            BS = range(BPC)
            xt = [st[b]["xt"] for b in BS]
            xa = [st[b]["xa"] for b in BS]
            AT = [st[b]["AT"] for b in BS]

            gcw = [[[ap.tile((CH[c][1], CH[c][1]), BF16, tag=f"gcw{b}_{c}_{s}",
                             bufs=2, name=f"gcw{b}_{i}_{c}_{s}")
                     for c in range(2)] for s in range(3)] for b in BS]
            for b in BS:
                for s in range(3):
                    for c in range(2):
                        nc.sync.dma_start(out=gcw[b][s][c][:],
                                          in_=d[f"wgc{c}"][i, s])

            # -- attention + sigmoid
            xn = [[None, None] for b in BS]
            r1 = [[None, None] for b in BS]
            sg = [[None, None] for b in BS]
            for b in BS:
                for c in range(2):
                    rows = CH[c][1]
                    r1[b][c] = ap.tile((rows, N), BF16, tag=f"R1{b}_{c}",
                                       name=f"R1{b}_{i}_{c}")
                    sg[b][c] = ap.tile((rows, N), F32, tag=f"tmp{b}_{c}",
                                       name=f"sg{b}_{i}_{c}")
                    xn[b][c] = ap.tile((rows, N), BF16, tag=f"XN{b}_{c}",
                                       name=f"XN{b}_{i}_{c}")
            for c in range(2):
                rows = CH[c][1]
                for nsi, ns in enumerate(NS):
                    m1s = []
                    for b in BS:
                        m1 = pp.tile((rows, 512), F32, tag="pwork", bufs=3,
                                     name=f"m1_{b}_{i}_{c}_{nsi}")
                        nc.tensor.matmul(m1[:], wfc1[c][:], xt[b][c][:, ns],
                                         start=True, stop=True)
                        m1s.append(m1)
                    for b in BS:
                        if b % 2 == 0:
                            nc.scalar.activation(r1[b][c][:, ns], m1s[b][:],
                                                 AF.Relu)
                        else:
                            nc.vector.tensor_scalar(r1[b][c][:, ns], m1s[b][:],
                                                    0.0, None, ALU.max)
                    aps = []
                    for b in BS:
                        a_ps = pp.tile((rows, 512), F32, tag="pwork", bufs=3,
                                       name=f"aps{b}_{i}_{c}_{nsi}")
                        nc.tensor.matmul(a_ps[:], wfc2[c][:], r1[b][c][:, ns],
                                         start=True, stop=True)
                        aps.append(a_ps)
                    for b in BS:
                        nc.vector.scalar_tensor_tensor(
                            sg[b][c][:, ns], aps[b][:], 2.0,
                            xt[b][c][:, ns].bitcast(F32), ALU.mult, ALU.add)
                    for b in BS:
                        nc.scalar.activation(xn[b][c][:, ns], sg[b][c][:, ns],
                                             AF.Sigmoid)

            # -- skip conv -> relu -> end1 matmul -> SBUF accumulator
            rsk = [ap.tile((SKR, N), BF16, tag=f"rsk{b}", name=f"rsk{b}_{i}")
                   for b in BS]
            for nsi, ns in enumerate(NS):
                sks = []
                for b in BS:
                    sk_ps = pp.tile((SKR, 512), F32, tag="pwork", bufs=3,
                                    name=f"skp{b}_{i}_{nsi}")
                    nc.tensor.matmul(sk_ps[:64], wskip[i][0][:],
                                     xn[b][0][:, ns], start=True, stop=True)
                    nc.tensor.matmul(sk_ps[64:], wskip[i][1][:],
                                     xn[b][1][:, ns], start=True, stop=True)
                    sks.append(sk_ps)
                for b in BS:
                    if b % 2 == 0:
                        nc.vector.tensor_scalar(rsk[b][:, ns], sks[b][:],
                                                vcol(f"skb{i}", SKR), 0.0,
                                                ALU.add, ALU.max)
                    else:
                        nc.scalar.activation(rsk[b][:, ns], sks[b][:], AF.Relu,
                                             bias=vcol(f"skb{i}", SKR))
                for b in BS:
                    e_ps = pp.tile((64, 512), F32, tag="pwork", bufs=3,
                                   name=f"eps{b}_{i}_{nsi}")
                    nc.tensor.matmul(e_ps[:], we1[i][:], rsk[b][:, ns],
                                     start=True, stop=True)
                    if i == 0:
                        nc.vector.tensor_copy(st[b]["end"][:, ns], e_ps[:])
                    else:
                        nc.vector.scalar_tensor_tensor(
                            st[b]["end"][:, ns], e_ps[:], 0.0,
                            st[b]["end"][:, ns], ALU.bypass, ALU.add)

            # -- V-layout of xn via PE transposes
            xv = [[None] * 8 for b in BS]
            for v in range(8):
                cs = slice(v * 128, (v + 1) * 128)
                for b in BS:
                    tp = pp.tile((128, CL), BF16, tag="ptr", bufs=3,
                                 name=f"tpx{b}_{i}_{v}")
                    for c in range(2):
                        o, rows = CH[c]
                        nc.tensor.transpose(tp[:, o:o + rows],
                                            xn[b][c][:, cs],
                                            idenb[:rows, :rows])
                    xv[b][v] = ap.tile((128, CL), BF16, tag=f"XV{b}_{v}",
                                       name=f"XV{b}_{i}_{v}")
                    nc.vector.tensor_copy(xv[b][v][:], tp[:])

            def hop(rv, nm):
                """A-hop (V-orientation, w-pairs) + transpose back, both b."""
                hvp = [[None] * 4 for b in BS]
                for p in range(4):
                    for b in BS:
                        h_ps = pp.tile((128, 2 * CL), F32, tag="ptr", bufs=3,
                                       name=f"hp{nm}{b}_{i}_{p}")
                        for half in range(2):
                            w = 2 * p + half
                            ws = slice(w * 128, (w + 1) * 128)
                            dst = h_ps[:, half * CL:(half + 1) * CL]
                            for k in range(8):
                                nc.tensor.matmul(dst, AT[b][k][:, ws], rv(b, k),
                                                 start=(k == 0), stop=(k == 7))
                        hvp[b][p] = ap.tile((128, 2 * CL), BF16,
                                            tag=f"{nm}V{b}_{p}",
                                            name=f"{nm}V{b}_{i}_{p}")
                        nc.vector.tensor_copy(hvp[b][p][:], h_ps[:])

                ht = [[ap.tile((CH[c][1], N), BF16, tag=f"{nm}T{b}_{c}",
                               name=f"{nm}T{b}_{i}_{c}") for c in range(2)]
                      for b in BS]
                for b in BS:
                    tpb = [pp.tile((CH[c][1], N), BF16, tag=f"ptb{c}",
                                   bufs=1, name=f"tpb{nm}{b}_{i}_{c}")
                           for c in range(2)]
                    for w in range(8):
                        src = hvp[b][w // 2][:, (w % 2) * CL:(w % 2) * CL + CL]
                        for c in range(2):
                            o, rows = CH[c]
                            nc.tensor.transpose(
                                tpb[c][:, w * 128:(w + 1) * 128],
                                src[:, o:o + rows], idenb[:, :])
                        if w % 4 == 3:
                            half = slice((w - 3) * 128, (w + 1) * 128)
                            for c in range(2):
                                if (b + c) % 2 == 0:
                                    nc.scalar.activation(ht[b][c][:, half],
                                                         tpb[c][:, half],
                                                         AF.Copy)
                                else:
                                    nc.vector.tensor_copy(ht[b][c][:, half],
                                                          tpb[c][:, half])
                return hvp, ht

            h1vp, h1t = hop(lambda b, k: xv[b][k][:], "H1")
            _, h2t = hop(
                lambda b, k: h1vp[b][k // 2][:, (k % 2) * CL:(k % 2) * CL + CL],
                "H2")

            # -- gconv (block-diag over l) + residuals + batchnorm
            for c in range(2):
                rows = CH[c][1]
                gt = [ap.tile((rows, N), F32, tag=f"tmp{b}_{c}",
                              name=f"gt{b}_{i}_{c}") for b in BS]
                nxt = [ap.tile((rows, N), F32R, tag=f"XT{b}_{c}", bufs=2,
                               name=f"XT{b}_{i}_{c}") for b in BS]
                for nsi, ns in enumerate(NS):
                    gps = []
                    for b in BS:
                        g_ps = pp.tile((rows, 512), F32, tag="pwork", bufs=3,
                                       name=f"gp{b}_{i}_{c}_{nsi}")
                        srcs = (xn[b], h1t[b], h2t[b])
                        for s in range(3):
                            nc.tensor.matmul(g_ps[:], gcw[b][s][c][:],
                                             srcs[s][c][:, ns],
                                             start=(s == 0), stop=(s == 2))
                        gps.append(g_ps)
                    for b in BS:
                        nc.vector.scalar_tensor_tensor(
                            gt[b][:, ns], xa[b][c][:, ns],
                            vcol(f"av{i}_{c}", rows), gps[b][:],
                            ALU.mult, ALU.add)
                    for b in BS:
                        nc.gpsimd.tensor_tensor(
                            gt[b][:, ns], gt[b][:, ns],
                            xt[b][c][:, ns].bitcast(F32), ALU.add)
                        nc.gpsimd.tensor_scalar(
                            nxt[b][:, ns], gt[b][:, ns],
                            vcol(f"bns{i}_{c}", rows),
                            vcol(f"bnb{i}_{c}", rows), ALU.mult, ALU.add)
                for b in BS:
                    xt[b][c] = nxt[b]

        # ---------------- end convs ----------------
        def tail(b):
            o1 = ap.tile((64, N), F32R, tag="o1", name=f"o1_{b}")
            ob = ap.tile((12, N), F32, tag="ob", name=f"ob{b}")
            for nsi, ns in enumerate(NS):
                nc.scalar.activation(o1[:, ns], st[b]["end"][:, ns], AF.Relu,
                                     bias=vcol("e1b", 64))
                o2_ps = pp.tile((12, 512), F32, tag="pwork", bufs=3,
                                name=f"o2p{b}_{nsi}")
                nc.tensor.matmul(o2_ps[:], we2[:], o1[:, ns],
                                 start=True, stop=True)
                nc.scalar.activation(ob[:, ns], o2_ps[:], AF.Identity,
                                     bias=vcol("e2b", 12))
            nc.sync.dma_start(out=outp[b], in_=ob[:])

        for b in range(BPC):
            phase0(b)
        for b in range(BPC):
            start(b)
        for i in range(L):
            layer_pair(i)
        for b in range(BPC):
            tail(b)

    nc.finalize()
    return nc


# ----------------------------------------------------------------------------
# host-side preprocessing
# ----------------------------------------------------------------------------

def _prep_host(inputs):
    f = lambda x: np.asarray(x, dtype=np.float32)
    bf = lambda x: np.ascontiguousarray(x).astype(ml_dtypes.bfloat16)
    x_in = f(inputs["inputs"])
    ind = np.asarray(inputs["ind"]).astype(np.int64)
    p1, p2, p3, pk = f(inputs["p1"]), f(inputs["p2"]), f(inputs["p3"]), f(inputs["pk"])

    xo = np.pad(x_in, ((0, 0), (0, 0), (0, 0), (RF - T, 0)))
    inp_t = np.ascontiguousarray(xo.transpose(0, 1, 3, 2))
    te = p1[ind]
    adp = np.einsum("bi,ijk->bjk", te, pk).astype(np.float32)

    start_w, start_b = f(inputs["start_w"]), f(inputs["start_b"])
    starta_w, starta_b = f(inputs["starta_w"]), f(inputs["starta_b"])
    fc1_w, fc2_w = f(inputs["fc1_w"]), f(inputs["fc2_w"])
    skip_w, skip_b = f(inputs["skip_w"]), f(inputs["skip_b"])
    gconv_w, gconv_b = f(inputs["gconv_w"]), f(inputs["gconv_b"])
    bn_g, bn_b = f(inputs["bn_g"]), f(inputs["bn_b"])
    bna_g, bna_b = f(inputs["bna_g"]), f(inputs["bna_b"])
    end1_w, end1_b = f(inputs["end1_w"]), f(inputs["end1_b"])
    end2_w, end2_b = f(inputs["end2_w"]), f(inputs["end2_b"])

    e8, e5 = np.eye(8, dtype=np.float32), np.eye(5, dtype=np.float32)
    e13 = np.eye(RF, dtype=np.float32)
    kr = lambda e, w: np.kron(e, np.ascontiguousarray(w.T)).astype(np.float32)

    wstart0 = np.stack([np.kron(e13[:, :8], w[:, 0][None, :])
                        for w in (start_w, starta_w)]).astype(np.float32)
    wstart1 = np.stack([np.kron(e13[:, 8:], w[:, 0][None, :])
                        for w in (start_w, starta_w)]).astype(np.float32)
    wgc0 = np.stack([np.stack([kr(e8, gconv_w[i][:, s * 16:(s + 1) * 16])
                               for s in range(3)]) for i in range(L)])
    wgc1 = np.stack([np.stack([kr(e5, gconv_w[i][:, s * 16:(s + 1) * 16])
                               for s in range(3)]) for i in range(L)])
    wskip0 = np.stack([kr(e8, skip_w[i]) for i in range(L)])
    wskip1 = np.stack([kr(e5, skip_w[i]) for i in range(L)])

    # end1 columns: ref skip rows are o*13+l within the (L-1-i)-th block;
    # ours are l*8+o
    we1 = np.zeros((L, SKR, 64), dtype=np.float32)
    ll, oo = np.meshgrid(np.arange(RF), np.arange(SC), indexing="ij")
    src_col = oo.ravel() * RF + ll.ravel()
    for i in range(L):
        we1[i] = end1_w[:, (L - 1 - i) * SKR + src_col].T

    t8 = lambda v: np.tile(v, 8)
    vecs = np.zeros((128, NV_COLS), dtype=np.float32)
    ci = 0
    vecs[:, ci] = t8(start_b); ci += 1
    vecs[:80, ci] = np.tile(start_b, 5); ci += 1
    vecs[:, ci] = t8(starta_b); ci += 1
    vecs[:80, ci] = np.tile(starta_b, 5); ci += 1
    for i in range(L):
        vecs[:SKR, ci] = np.tile(skip_b[i], RF); ci += 1
    bns = (bn_g / np.sqrt(1.0 + BN_EPS)).astype(np.float32)
    bnas = (bna_g / np.sqrt(1.0 + BN_EPS)).astype(np.float32)
    av = np.ones(16, dtype=np.float32)
    bv = np.zeros(16, dtype=np.float32)
    for i in range(L):
        bnb_adj = bn_b[i] + bns[i] * (gconv_b[i] + bv)
        vecs[:, ci] = t8(bns[i]); ci += 1
        vecs[:, ci] = t8(bnb_adj); ci += 1
        vecs[:, ci] = t8(av); ci += 1
        vecs[:80, ci] = np.tile(bns[i], 5); ci += 1
        vecs[:80, ci] = np.tile(bnb_adj, 5); ci += 1
        vecs[:80, ci] = np.tile(av, 5); ci += 1
        av = 2.0 * bnas[i] * av
        bv = 2.0 * bnas[i] * bv + bna_b[i]
    vecs[:64, ci] = end1_b; ci += 1
    vecs[:12, ci] = end2_b; ci += 1
    assert ci == NV_COLS

    shared = {
        "p2T": np.ascontiguousarray(p2.T),
        "p3sT": np.ascontiguousarray(p3[:DIMS, :DIMS].T),
        "wstart0": wstart0, "wstart1": wstart1,
        "wfc1_0": kr(e8, fc1_w), "wfc1_1": kr(e5, fc1_w),
        "wfc2_0": bf(kr(e8, fc2_w)), "wfc2_1": bf(kr(e5, fc2_w)),
        "wskip0": bf(wskip0), "wskip1": bf(wskip1),
        "wgc0": bf(wgc0), "wgc1": bf(wgc1),
        "we1": bf(we1), "we2": np.ascontiguousarray(end2_w.T),
        "idenb": np.eye(128, dtype=ml_dtypes.bfloat16),
        "vecs": vecs,
    }
    in_maps = []
    for c in range(NCORES):
        bs = slice(c * BPC, (c + 1) * BPC)
        m = dict(shared)
        m["inp"] = np.ascontiguousarray(inp_t[bs])
        m["adp"] = np.ascontiguousarray(adp[bs])
        in_maps.append(m)
    return in_maps


def _get_nc():
    global _CACHED
    if _CACHED is None:
        _CACHED = _build_nc()
    return _CACHED


def run(inputs, trace=False):
    nc = _get_nc()
    in_maps = _prep_host(inputs)
    res = run_bass_kernel_spmd(nc, in_maps, core_ids=list(range(NCORES)),
                               trace=trace)
    out = np.stack([res.results[c]["outp"] for c in range(NCORES)])
    out = out.reshape(B, 12, N, 1).astype(np.float32)
    return out, res


def kernel(**inputs):
    out, _ = run(inputs)
    return out
